# revision 42
# baseline (speedup 1.0000x reference)
"""DeepseekV2 MLA attention on 8 Trainium2 NeuronCores (Bass/Tile), v5.

All-bf16 datapath (fp32 PSUM).  Token-sharded front end computes the q/kv
latents on its 256-token shard; the 576-row kv latent (normalized kv_a +
roped k_pe) is AllGathered early so each core expands k_nope/v for its own
2 heads over all 2048 tokens while the q_b outputs (all heads, own shard)
are AllToAll'd.  Attention + row-parallel w_o as in v4; causal mask is
applied additively in PSUM via an identity matmul; host sums partials.
"""

import numpy as np
import ml_dtypes

import concourse.bass as bass
import concourse.bacc as bacc
import concourse.mybir as mybir
import concourse.tile as tile
from concourse import bass_utils

T = 2048
HID = 2048
H = 16
DN = 128
DR = 64
DV = 128
DQK = DN + DR
QLR = 1536
KVLR = 512
THETA = 10000.0
EPS = 1e-6
SCALE = DQK ** -0.5

NCORES = 8
HPC = H // NCORES
LATR = KVLR + DR          # 576 rows of exchanged kv latent

F32 = mybir.dt.float32
BF = mybir.dt.bfloat16
F8 = mybir.dt.float8e4
BF_NP = ml_dtypes.bfloat16
F8_NP = ml_dtypes.float8_e4m3

Q8 = True                 # exchange q_b outputs in fp8e4m3
QE = F8 if Q8 else BF
QE_NP = F8_NP if Q8 else BF_NP

KT = HID // 128           # 16 contraction strips over hidden
QMT = QLR // 128          # 12
KVMT = KVLR // 128        # 4
NB = T // 512             # 4 query blocks
TBT = T // 128            # 16 token blocks
TSH = T // NCORES         # 256 tokens per shard

QCH = 3 * 128             # 384 rows per dest in the q exchange
MASKV = -60.0


def build_bass():
    nc = bacc.Bacc(
        "TRN2",
        target_bir_lowering=False,
        debug=False,
        enable_asserts=False,
        num_devices=NCORES,
    )

    hs_sh = nc.dram_tensor("hs_sh", [HID, TSH], BF, kind="ExternalInput").ap()
    wqa = nc.dram_tensor("wqa", [QMT * 128, KT * 128], BF, kind="ExternalInput").ap()
    wkva = nc.dram_tensor("wkva", [KVMT * 128, KT * 128], BF, kind="ExternalInput").ap()
    wkpe = nc.dram_tensor("wkpe", [128, KT * DR], BF, kind="ExternalInput").ap()
    wqb = nc.dram_tensor("wqb", [NCORES * 128, QMT * QCH], BF, kind="ExternalInput").ap()
    wkvb = nc.dram_tensor("wkvb", [128, KVMT * 4 * 128], BF, kind="ExternalInput").ap()
    wo = nc.dram_tensor("wo", [HPC * DV, HID], BF, kind="ExternalInput").ap()
    cosf2 = nc.dram_tensor("cosf2", [128, TSH], BF, kind="ExternalInput").ap()
    sinf2 = nc.dram_tensor("sinf2", [128, TSH], BF, kind="ExternalInput").ap()
    perm128 = nc.dram_tensor("perm128", [128, 128], BF, kind="ExternalInput").ap()
    selswap = nc.dram_tensor("selswap", [128, 128], BF, kind="ExternalInput").ap()
    ident = nc.dram_tensor("ident", [128, 128], BF, kind="ExternalInput").ap()
    maskd = nc.dram_tensor("maskd", [128, 4 * 512], BF, kind="ExternalInput").ap()
    ones = nc.dram_tensor("ones", [128, 128], BF, kind="ExternalInput").ap()
    out = nc.dram_tensor("out", [T, HID], BF, kind="ExternalOutput").ap()

    with tile.TileContext(nc) as tc:
        _kernel_body(nc, tc, hs_sh, wqa, wkva, wkpe, wqb, wkvb, wo,
                     cosf2, sinf2, perm128, selswap, ident, maskd, ones, out)

    nc.compile()
    return nc


def _kernel_body(nc, tc, hs_sh, wqa, wkva, wkpe, wqb, wkvb, wo,
                 cosf2, sinf2, perm128, selswap, ident, maskd, ones, out):
    from contextlib import ExitStack

    ctx = ExitStack()
    with ctx:
        dram = ctx.enter_context(tc.tile_pool(name="dram", bufs=1, space="DRAM"))
        contrib_kv = dram.tile([LATR, TSH], BF)
        a2a_kv = dram.tile([NCORES * LATR, TSH], BF)
        QHC = DN + DR          # 192 rows per dest per head
        contrib_qh = [dram.tile([NCORES * QHC, TSH], QE, name=f"cq{h}")
                      for h in range(HPC)]
        a2a_qh = [dram.tile([NCORES * QHC, TSH], QE, name=f"aq{h}")
                  for h in range(HPC)]

        persist = ctx.enter_context(tc.tile_pool(name="persist", bufs=1))
        ident_t = persist.tile([128, 128], BF, tag="ident")
        nc.gpsimd.dma_start(out=ident_t, in_=ident)
        perm_t = persist.tile([128, 128], BF, tag="perm")
        nc.gpsimd.dma_start(out=perm_t, in_=perm128)
        selswap_t = persist.tile([128, 128], BF, tag="selswap")
        nc.gpsimd.dma_start(out=selswap_t, in_=selswap)
        cos_t = persist.tile([128, TSH], BF, tag="cos")
        nc.gpsimd.dma_start(out=cos_t, in_=cosf2)
        sin_t = persist.tile([128, TSH], BF, tag="sin")
        nc.gpsimd.dma_start(out=sin_t, in_=sinf2)
        ones_t = persist.tile([128, 128], BF, tag="ones")
        nc.gpsimd.dma_start(out=ones_t, in_=ones)
        # q_b weights for the first dests ride the otherwise-idle Pool queue
        wq_t = []
        for d in range(NCORES):
            wq_t.append(persist.tile([128, QMT * QCH], BF, tag=f"wq{d}",
                                     name=f"wq{d}"))
        for d in range(4):
            nc.gpsimd.dma_start(out=wq_t[d], in_=wqb[d * 128:(d + 1) * 128, :])
        maskd_t = persist.tile([128, 4 * 512], BF, tag="maskd")
        nc.gpsimd.dma_start(out=maskd_t, in_=maskd)
        wkvb_t = persist.tile([128, KVMT, 4 * 128], BF, tag="wkvb")
        nc.gpsimd.dma_start(
            out=wkvb_t, in_=wkvb.rearrange("p (s c) -> p s c", s=KVMT))
        wo_t = []
        for h in range(HPC):
            w = persist.tile([128, HID], BF, tag=f"wo{h}")
            nc.gpsimd.dma_start(out=w, in_=wo[h * DV:(h + 1) * DV, :])
            wo_t.append(w)
        ones_col = ones_t[:, 0:1]
        ones_row = ones_t[0:1, :]

        pmid = ctx.enter_context(tc.tile_pool(name="pmid", bufs=1))

        # ---- Phase A: latents on own shard --------------------------------
        with tc.tile_pool(name="pa", bufs=1) as pa, \
             tc.tile_pool(name="psa", bufs=1, space="PSUM") as psa:
            # kv_a weights first so the kv latent (and its AllGather) start
            # as early as possible; hidden states come in one fat DMA.
            wkva0_t = pa.tile([128, KT * 128], BF, tag="wkva0")
            nc.sync.dma_start(out=wkva0_t, in_=wkva[0:128, :])
            hs_t = pa.tile([128, KT, TSH], BF, tag="hst")
            wkva123_t = pa.tile([128, 3, KT * 128], BF, tag="wkva123")
            nc.sync.dma_start(
                out=hs_t[:, 0:KT // 2, :],
                in_=hs_sh[0:HID // 2].rearrange("(k p) t -> p k t", p=128))
            nc.sync.dma_start(
                out=wkva123_t[:, 0, :], in_=wkva[128:256, :])
            nc.sync.dma_start(
                out=hs_t[:, KT // 2:, :],
                in_=hs_sh[HID // 2:].rearrange("(k p) t -> p k t", p=128))
            nc.sync.dma_start(out=wkva123_t[:, 1, :], in_=wkva[256:384, :])
            nc.sync.dma_start(out=wkva123_t[:, 2, :], in_=wkva[384:, :])
            hst = [hs_t[:, k, :] for k in range(KT)]
            wkva_t = [wkva0_t] + [wkva123_t[:, m, :] for m in range(3)]
            wkpe_t = pa.tile([128, KT * DR], BF, tag="wkpe")
            nc.sync.dma_start(out=wkpe_t, in_=wkpe)

            def rsqrt_bc(z_psum, n, tag):
                tmp = pa.tile([1, TSH], F32, tag="rsq_tmp", bufs=2)
                nc.scalar.activation(tmp, z_psum,
                                     mybir.ActivationFunctionType.Copy,
                                     bias=EPS, scale=1.0 / n)
                nc.vector.reciprocal(tmp, tmp)
                srow = pa.tile([1, TSH], BF, tag=tag + "r", name=tag + "r")
                nc.scalar.activation(srow, tmp,
                                     mybir.ActivationFunctionType.Sqrt)
                b_ps = psa.tile([128, TSH], F32, tag="bc", bufs=1)
                nc.tensor.matmul(b_ps, lhsT=ones_row, rhs=srow,
                                 start=True, stop=True)
                bc = pmid.tile([128, TSH], BF, tag=tag, name=tag)
                nc.scalar.copy(bc, b_ps)
                return bc

            zkv = psa.tile([1, TSH], F32, tag="zkv")
            kv_raw = []   # bf16 un-normalized latent strips
            for m in range(KVMT):
                pq = psa.tile([128, TSH], F32, tag="pq", bufs=3)
                for k in range(KT):
                    nc.tensor.matmul(pq, lhsT=wkva_t[m][:, k * 128:(k + 1) * 128],
                                     rhs=hst[k],
                                     start=(k == 0), stop=(k == KT - 1))
                st = pa.tile([128, TSH], BF, tag=f"kvr{m}", name=f"kvr{m}")
                nc.vector.tensor_copy(st, pq)
                kv_raw.append(st)
                sq = pa.tile([128, TSH], BF, tag="sq", bufs=2)
                nc.vector.tensor_tensor(sq, st, st, op=mybir.AluOpType.mult)
                nc.tensor.matmul(zkv, lhsT=ones_col, rhs=sq,
                                 start=(m == 0), stop=(m == KVMT - 1))
            # raw k_pe
            kpe_ps = psa.tile([DR, TSH], F32, tag="kpeps")
            for k in range(KT):
                nc.tensor.matmul(kpe_ps, lhsT=wkpe_t[:, k * DR:(k + 1) * DR],
                                 rhs=hst[k],
                                 start=(k == 0), stop=(k == KT - 1))
            kpe_raw = pa.tile([DR, TSH], BF, tag="kperaw")
            nc.vector.tensor_copy(kpe_raw, kpe_ps)

            skv_bc = rsqrt_bc(zkv, KVLR, "skvbc")
            # normalized latent staged contiguously for one contrib DMA
            kvstage = pa.tile([128, KVMT, TSH], BF, tag="kvstage")
            for m in range(KVMT):
                nc.vector.tensor_tensor(kvstage[:, m, :], kv_raw[m], skv_bc,
                                        op=mybir.AluOpType.mult)
            # rope k_pe (64 rows; use top half of perm/cos/sin)
            sw_ps = psa.tile([DR, TSH], F32, tag="swk")
            nc.tensor.matmul(sw_ps, lhsT=perm_t[0:DR, 0:DR], rhs=kpe_raw,
                             start=True, stop=True)
            rt1 = pa.tile([DR, TSH], BF, tag="rt1")
            nc.vector.tensor_tensor(rt1, kpe_raw, cos_t[0:DR, :],
                                    op=mybir.AluOpType.mult)
            rt2 = pa.tile([DR, TSH], BF, tag="rt2")
            nc.vector.tensor_tensor(rt2, sw_ps, sin_t[0:DR, :],
                                    op=mybir.AluOpType.mult)
            kpel = pa.tile([DR, TSH], BF, tag="kpel")
            nc.vector.tensor_tensor(kpel, rt1, rt2, op=mybir.AluOpType.add)

            nc.gpsimd.dma_start(
                out=contrib_kv[0:KVLR, :].rearrange("(g p) t -> p g t", p=128),
                in_=kvstage)
            nc.gpsimd.dma_start(out=contrib_kv[KVLR:LATR, :], in_=kpel)
            nc.gpsimd.collective_compute(
                "AllGather", mybir.AluOpType.bypass,
                replica_groups=[list(range(NCORES))],
                ins=[contrib_kv], outs=[a2a_kv])

            # q latent
            zq = psa.tile([1, TSH], F32, tag="zq")
            q_raw = []
            for m in range(QMT):
                wt = pa.tile([128, KT * 128], BF, tag="wqa", bufs=4)
                weng = nc.sync if m % 2 == 0 else nc.scalar
                weng.dma_start(out=wt, in_=wqa[m * 128:(m + 1) * 128, :])
                pq = psa.tile([128, TSH], F32, tag="pq", bufs=3)
                for k in range(KT):
                    nc.tensor.matmul(pq, lhsT=wt[:, k * 128:(k + 1) * 128],
                                     rhs=hst[k],
                                     start=(k == 0), stop=(k == KT - 1))
                st = pmid.tile([128, TSH], BF, tag=f"qr{m}", name=f"qr{m}")
                nc.vector.tensor_copy(st, pq)
                q_raw.append(st)
                sq = pa.tile([128, TSH], BF, tag="sq", bufs=2)
                nc.vector.tensor_tensor(sq, st, st, op=mybir.AluOpType.mult)
                nc.tensor.matmul(zq, lhsT=ones_col, rhs=sq,
                                 start=(m == 0), stop=(m == QMT - 1))
            sq_bc = rsqrt_bc(zq, QLR, "sqbc")
            qan = []
            for m in range(QMT):
                qq = pmid.tile([128, TSH], BF, tag=f"qan{m}", name=f"qan{m}")
                nc.vector.tensor_tensor(qq, q_raw[m], sq_bc,
                                        op=mybir.AluOpType.mult)
                qan.append(qq)

        # ---- q_b for all dests + per-head exchange ------------------------
        # head-0 AllToAll goes first so head-0 attention can overlap the
        # head-1 AllToAll.
        with tc.tile_pool(name="pw", bufs=1) as pw, \
             tc.tile_pool(name="psw", bufs=1, space="PSUM") as psw:
            st_qn = [pw.tile([128, NCORES, TSH], QE, tag=f"stqn{h}",
                             name=f"stqn{h}") for h in range(HPC)]
            st_pe = [pw.tile([DR, NCORES, TSH], QE, tag=f"stpe{h}",
                             name=f"stpe{h}") for h in range(HPC)]
            for d in range(4, NCORES):
                nc.scalar.dma_start(out=wq_t[d],
                                    in_=wqb[d * 128:(d + 1) * 128, :])
            cos64 = cos_t[0:DR, :]
            sin64 = sin_t[0:DR, :]
            for d in range(NCORES):
                wq = wq_t[d]
                accq = []
                for mt in range(3):
                    a = psw.tile([128, TSH], F32, tag="acc", bufs=4,
                                 name=f"accq{mt}")
                    accq.append(a)
                for k in range(QMT):
                    for mt in range(3):
                        nc.tensor.matmul(
                            accq[mt],
                            lhsT=wq[:, k * QCH + mt * 128:k * QCH + (mt + 1) * 128],
                            rhs=qan[k],
                            start=(k == 0), stop=(k == QMT - 1))
                for hh in range(HPC):
                    nc.vector.tensor_copy(st_qn[hh][:, d, :], accq[hh])
                # q_pe rope, heads split to base-0 64-row tiles
                qraw = pw.tile([128, TSH], BF, tag="qraw", bufs=2)
                nc.vector.tensor_copy(qraw, accq[2])
                rope3 = psw.tile([DR, 3, TSH], F32, tag="rope3", bufs=2)
                sw0, raw1, sw1 = rope3[:, 0, :], rope3[:, 1, :], rope3[:, 2, :]
                nc.tensor.matmul(sw0, lhsT=perm_t[:, 0:DR], rhs=qraw,
                                 start=True, stop=True)
                nc.tensor.matmul(raw1, lhsT=selswap_t[:, 0:DR], rhs=qraw,
                                 start=True, stop=True)
                nc.tensor.matmul(sw1, lhsT=selswap_t[:, DR:2 * DR], rhs=qraw,
                                 start=True, stop=True)
                r1 = pw.tile([DR, TSH], BF, tag="r1", bufs=4)
                nc.vector.tensor_tensor(r1, qraw[0:DR, :], cos64,
                                        op=mybir.AluOpType.mult)
                r2 = pw.tile([DR, TSH], BF, tag="r2", bufs=4)
                nc.vector.tensor_tensor(r2, sw0, sin64,
                                        op=mybir.AluOpType.mult)
                nc.vector.tensor_tensor(st_pe[0][:, d, :], r1, r2,
                                        op=mybir.AluOpType.add)
                r1b = pw.tile([DR, TSH], BF, tag="r1b", bufs=4)
                nc.vector.tensor_tensor(r1b, raw1, cos64,
                                        op=mybir.AluOpType.mult)
                r2b = pw.tile([DR, TSH], BF, tag="r2b", bufs=4)
                nc.vector.tensor_tensor(r2b, sw1, sin64,
                                        op=mybir.AluOpType.mult)
                nc.vector.tensor_tensor(st_pe[1][:, d, :], r1b, r2b,
                                        op=mybir.AluOpType.add)
            QHC = DN + DR
            for h in range(HPC):
                nc.sync.dma_start(
                    out=contrib_qh[h].rearrange("(d c) t -> c d t",
                                                d=NCORES)[0:DN],
                    in_=st_qn[h])
                nc.sync.dma_start(
                    out=contrib_qh[h].rearrange("(d c) t -> c d t",
                                                d=NCORES)[DN:QHC],
                    in_=st_pe[h])
                nc.gpsimd.collective_compute(
                    "AllToAll", mybir.AluOpType.bypass,
                    replica_groups=[list(range(NCORES))],
                    ins=[contrib_qh[h]], outs=[a2a_qh[h]])

        # ---- Phase B: expand k_nope / v for own heads over all tokens -----
        bcp = ctx.enter_context(tc.tile_pool(name="bcp", bufs=1))
        # NOTE: keep these off the gpsimd queue — instructions behind a
        # collective on the same queue only run after the collective ends.
        kvan = []      # latent strips, all tokens [128, 8, 256]
        for r in range(KVMT):
            kt_ = bcp.tile([128, NCORES, TSH], BF, tag=f"kvan{r}",
                           name=f"kvan{r}")
            eng = nc.sync if r % 2 == 0 else nc.scalar
            eng.dma_start(
                out=kt_,
                in_=a2a_kv.rearrange("(s r) t -> r s t", s=NCORES)
                            [r * 128:(r + 1) * 128])
            kvan.append(kt_)
        kpe_all = bcp.tile([DR, NCORES, TSH], BF, tag="kpe")
        nc.scalar.dma_start(
            out=kpe_all,
            in_=a2a_kv.rearrange("(s r) t -> r s t", s=NCORES)[KVLR:LATR])

        def tok512(tile3, c):
            # 512-token chunk c of a [*, 8, 256] tile
            return tile3[:, 2 * c:2 * c + 2, :]

        def tok128(tile3, tb):
            half = (tb % 2) * 128
            return tile3[:, tb // 2, half:half + 128]

        kn = []        # per head [128, 8, 256] feature-major k_nope
        vt = [None] * TBT   # per 128-token block [128, HPC*DV] token-major v
        with tc.tile_pool(name="pb", bufs=1) as pb, \
             tc.tile_pool(name="psb", bufs=1, space="PSUM") as psb:
            for h in range(HPC):
                knh = bcp.tile([128, NCORES, TSH], BF, tag=f"kn{h}",
                               name=f"kn{h}")
                for c in range(4):
                    acck = psb.tile([128, 512], F32, tag="acck", bufs=2)
                    for s in range(KVMT):
                        nc.tensor.matmul(
                            acck, lhsT=wkvb_t[:, s, h * DN:(h + 1) * DN],
                            rhs=tok512(kvan[s], c),
                            start=(s == 0), stop=(s == KVMT - 1))
                    nc.vector.tensor_copy(tok512(knh, c), acck)
                kn.append(knh)
            for tb in range(TBT):
                accv = psb.tile([128, HPC * DV], F32, tag="accv", bufs=3)
                for s in range(KVMT):
                    nc.tensor.matmul(
                        accv, lhsT=tok128(kvan[s], tb),
                        rhs=wkvb_t[:, s, 2 * DN:2 * DN + HPC * DV],
                        start=(s == 0), stop=(s == KVMT - 1))
                vt[tb] = bcp.tile([128, HPC * DV], BF, tag=f"v{tb}",
                                  name=f"v{tb}")
                nc.vector.tensor_copy(vt[tb], accv)

        # q tiles for own heads, all tokens
        # per-head receive: head h tiles land right after its AllToAll; keep
        # them on SP so the Act queue stays free for head-0 attention exp.
        QHC = DN + DR
        qn = []    # [h][half] tiles of 4 source shards each
        qpe = []
        for h in range(HPC):
            qnh, qpeh = [], []
            for sh in range(2):
                ssl = slice(sh * 4, sh * 4 + 4)
                qh = bcp.tile([128, 4, TSH], QE, tag=f"qn{h}_{sh}",
                              name=f"qn{h}_{sh}")
                nc.sync.dma_start(
                    out=qh,
                    in_=a2a_qh[h].rearrange("(s c) t -> c s t",
                                            s=NCORES)[0:DN, ssl])
                qnh.append(qh)
                qp = bcp.tile([DR, 4, TSH], QE, tag=f"qpe{h}_{sh}",
                              name=f"qpe{h}_{sh}")
                nc.sync.dma_start(
                    out=qp,
                    in_=a2a_qh[h].rearrange("(s c) t -> c s t",
                                            s=NCORES)[DN:QHC, ssl])
                qpeh.append(qp)
            qn.append(qnh)
            qpe.append(qpeh)

        def q512(tiles, qj):
            # 512-token query chunk qj from the source-split tiles
            return tiles[qj // 2][:, (qj % 2) * 2:(qj % 2) * 2 + 2, :]

        # ---- Attention + output projection --------------------------------
        with tc.tile_pool(name="pc", bufs=1) as pc, \
             tc.tile_pool(name="psc", bufs=1, space="PSUM") as psc:
            import concourse.bass_isa as bass_isa
            attn_n = [[None] * NB for _ in range(HPC)]
            for h in range(HPC):
                for qj in range(NB):
                    nki = 4 * qj + 4
                    attn_ps = psc.tile([128, 512], F32, tag="attn", bufs=2)
                    use_zmm = (h == 0 and qj < 2)
                    zrow_ps = None
                    z_acc = None
                    if use_zmm:
                        zrow_ps = psc.tile([1, 512], F32, tag="zrow", bufs=1)
                    else:
                        z_acc = pc.tile([128, 512], BF, tag="zacc", bufs=4)
                    for ki in range(nki):
                        s_ps = psc.tile([128, 512], F32, tag="s", bufs=3)
                        diag = ki >= 4 * qj
                        sub = ki - 4 * qj if diag else 0
                        if diag and sub >= 2:
                            # masked-out left part is a closed group of its
                            # own; narrow the score matmuls to the window
                            nc.tensor.matmul(
                                s_ps[:, 0:sub * 128], lhsT=ident_t,
                                rhs=maskd_t[:, sub * 512:sub * 512 + sub * 128],
                                start=True, stop=True)
                            sw = s_ps[:, sub * 128:]
                            nc.tensor.matmul(
                                sw, lhsT=ident_t,
                                rhs=maskd_t[:, sub * 512 + sub * 128:
                                            (sub + 1) * 512],
                                start=True, stop=False)
                            qn_w = q512(qn[h], qj)[:, 1, (sub - 2) * 128:]
                            qpe_w = q512(qpe[h], qj)[:, 1, (sub - 2) * 128:]
                            nc.tensor.matmul(sw, lhsT=tok128(kn[h], ki),
                                             rhs=qn_w, start=False, stop=False)
                            nc.tensor.matmul(sw, lhsT=tok128(kpe_all, ki),
                                             rhs=qpe_w, start=False, stop=True)
                        else:
                            nc.tensor.matmul(s_ps, lhsT=tok128(kn[h], ki),
                                             rhs=q512(qn[h], qj),
                                             start=True, stop=False)
                            nc.tensor.matmul(s_ps, lhsT=tok128(kpe_all, ki),
                                             rhs=q512(qpe[h], qj),
                                             start=False, stop=not diag)
                            if diag:
                                nc.tensor.matmul(
                                    s_ps, lhsT=ident_t,
                                    rhs=maskd_t[:, sub * 512:(sub + 1) * 512],
                                    start=False, stop=True)
                        e = pc.tile([128, 512], BF, tag="e", bufs=30)
                        nc.scalar.activation(e, s_ps,
                                             mybir.ActivationFunctionType.Exp)
                        if use_zmm:
                            # Pool is running collectives; z on the PE
                            nc.tensor.matmul(zrow_ps, lhsT=ones_col, rhs=e,
                                             start=(ki == 0),
                                             stop=(ki == nki - 1))
                        else:
                            # Pool is free now; z on the Pool engine
                            zt = pc.tile([128, 512], BF, tag="zt", bufs=3)
                            dst = z_acc if ki == 0 else zt
                            nc.gpsimd.partition_all_reduce(
                                dst, e, channels=128,
                                reduce_op=bass_isa.ReduceOp.add)
                            if ki > 0:
                                nc.vector.tensor_tensor(
                                    z_acc, z_acc, zt, op=mybir.AluOpType.add)
                        nc.tensor.matmul(attn_ps,
                                         lhsT=vt[ki][:, h * DV:(h + 1) * DV],
                                         rhs=e,
                                         start=(ki == 0), stop=(ki == nki - 1))
                    if use_zmm:
                        zr = pc.tile([1, 512], BF, tag="zr", bufs=2)
                        with nc.allow_low_precision(reason="bf16 softmax"):
                            nc.vector.tensor_copy(zr, zrow_ps)
                        zb_ps = psc.tile([128, 512], F32, tag="o", bufs=2,
                                          name="zb_ps")
                        nc.tensor.matmul(zb_ps, lhsT=ones_row, rhs=zr,
                                         start=True, stop=True)
                        z_acc = pc.tile([128, 512], BF, tag="zacc2", bufs=2)
                        nc.scalar.copy(z_acc, zb_ps)
                    attn_n[h][qj] = bcp.tile([128, 512], BF,
                                             tag=f"attn{h}_{qj}",
                                             name=f"attn{h}_{qj}")
                    rzb = pc.tile([128, 512], BF, tag="rzb", bufs=2)
                    with nc.allow_low_precision(reason="bf16 softmax"):
                        nc.vector.reciprocal(rzb, z_acc)
                    nc.vector.tensor_tensor(attn_n[h][qj], attn_ps, rzb,
                                            op=mybir.AluOpType.mult)

                    if h == HPC - 1:
                        # both heads' attn_n for this query block are ready
                        for tt in range(4):
                            tb = qj * 4 + tt
                            tsl = slice(tt * 128, (tt + 1) * 128)
                            o_row = pc.tile([128, HID], BF, tag="orow",
                                            bufs=2)
                            for hb in range(NB):
                                o_ps = psc.tile([128, 512], F32, tag="o",
                                                bufs=2)
                                for hh in range(HPC):
                                    nc.tensor.matmul(
                                        o_ps,
                                        lhsT=attn_n[hh][qj][:, tsl],
                                        rhs=wo_t[hh][:, hb * 512:(hb + 1) * 512],
                                        start=(hh == 0),
                                        stop=(hh == HPC - 1),
                                    )
                                osl = o_row[:, hb * 512:(hb + 1) * 512]
                                if hb % 2 == 0:
                                    nc.vector.tensor_copy(osl, o_ps)
                                else:
                                    nc.scalar.copy(osl, o_ps)
                                if tb == TBT - 1:
                                    weng = nc.sync if hb % 2 == 0 else nc.scalar
                                    weng.dma_start(
                                        out=out[tb * 128:(tb + 1) * 128,
                                                hb * 512:(hb + 1) * 512],
                                        in_=osl)
                            if tb < TBT - 1:
                                nc.sync.dma_start(
                                    out=out[tb * 128:(tb + 1) * 128, :],
                                    in_=o_row)


_NC_CACHE = {}


def _get_nc():
    if "nc" not in _NC_CACHE:
        _NC_CACHE["nc"] = build_bass()
    return _NC_CACHE["nc"]


def make_in_maps(positions, hidden_states, w_q_a, q_a_ln_w, w_q_b, w_kv_a,
                 kv_a_ln_w, w_kv_b, w_o):
    positions = np.asarray(positions)
    hidden_states = np.asarray(hidden_states, dtype=np.float32)
    w_q_a = np.asarray(w_q_a, dtype=np.float32)
    q_a_ln_w = np.asarray(q_a_ln_w, dtype=np.float32)
    w_q_b = np.asarray(w_q_b, dtype=np.float32)
    w_kv_a = np.asarray(w_kv_a, dtype=np.float32)
    kv_a_ln_w = np.asarray(kv_a_ln_w, dtype=np.float32)
    w_kv_b = np.asarray(w_kv_b, dtype=np.float32)
    w_o = np.asarray(w_o, dtype=np.float32)

    hs_t = np.ascontiguousarray(hidden_states.T)

    order = np.concatenate([np.arange(0, DR, 2), np.arange(1, DR, 2)])

    wkva_p = w_kv_a.copy()
    wkva_p[:, KVLR:] = w_kv_a[:, KVLR:][:, order]

    inv_freq = 1.0 / (THETA ** (np.arange(0, DR, 2, dtype=np.float64) / DR))
    ang = positions.astype(np.float64)[:, None] * inv_freq[None, :]
    cosT = np.cos(ang).T.astype(np.float32)
    sinT = np.sin(ang).T.astype(np.float32)
    cosf = np.concatenate([cosT, cosT], axis=0)          # [64, T]
    sinf = np.concatenate([-sinT, sinT], axis=0)
    cosf2 = np.concatenate([cosf, cosf], axis=0)         # [128, T] two heads
    sinf2 = np.concatenate([sinf, sinf], axis=0)

    perm = np.zeros((DR, DR), dtype=np.float32)
    for i in range(DR):
        perm[i, (i + DR // 2) % DR] = 1.0
    perm128 = np.zeros((128, 128), dtype=np.float32)
    perm128[:DR, :DR] = perm
    perm128[DR:, DR:] = perm
    selswap = np.zeros((128, 128), dtype=np.float32)
    for i in range(DR):
        selswap[DR + i, i] = 1.0                      # extract h1 raw
        selswap[DR + (i + DR // 2) % DR, DR + i] = 1.0  # extract h1 swapped

    # additive causal mask for the 4 diagonal sub-positions
    maskd = np.zeros((128, 4 * 512), dtype=np.float32)
    p = np.arange(128)[:, None]
    f = np.arange(512)[None, :]
    for sub in range(4):
        maskd[:, sub * 512:(sub + 1) * 512] = np.where(
            p + 128 * sub <= f, 0.0, MASKV)

    # q_b columns per dest: [qn_h0 | qn_h1 | qpe_h0(perm) ; qpe_h1(perm)]
    wqb_all = np.concatenate([
        np.concatenate([
            w_q_b[:, h0 * DQK:h0 * DQK + DN],
            w_q_b[:, h1 * DQK:h1 * DQK + DN],
            w_q_b[:, h0 * DQK + DN:(h0 + 1) * DQK][:, order],
            w_q_b[:, h1 * DQK + DN:(h1 + 1) * DQK][:, order],
        ], axis=1)
        for h0, h1 in ((2 * d, 2 * d + 1) for d in range(NCORES))
    ], axis=1) * q_a_ln_w[:, None] * SCALE

    def pack(w, mrows):
        Kd, Md = w.shape
        n = Md // mrows
        return np.ascontiguousarray(
            w.reshape(Kd // 128, 128, n, mrows).transpose(2, 1, 0, 3)
            .reshape(n * 128, (Kd // 128) * mrows))

    wqa_pk = pack(w_q_a, 128)
    wkva_pk = pack(wkva_p[:, :KVLR], 128)
    wkpe_pk = pack(wkva_p[:, KVLR:], DR)
    wqb_pk = pack(wqb_all, QCH)

    def bf(x):
        return np.ascontiguousarray(np.asarray(x, dtype=np.float32)).astype(BF_NP)

    in_maps = []
    for c in range(NCORES):
        h0, h1 = HPC * c, HPC * c + 1
        # own-head kv_b columns: [kn_h0 | kn_h1 | v_h0 | v_h1], ln folded
        wkvb_own = np.concatenate([
            w_kv_b[:, h0 * (DN + DV):h0 * (DN + DV) + DN],
            w_kv_b[:, h1 * (DN + DV):h1 * (DN + DV) + DN],
            w_kv_b[:, h0 * (DN + DV) + DN:(h0 + 1) * (DN + DV)],
            w_kv_b[:, h1 * (DN + DV) + DN:(h1 + 1) * (DN + DV)],
        ], axis=1) * kv_a_ln_w[:, None]
        wkvb_pk = pack(wkvb_own, 4 * 128)
        wo_c = np.concatenate([
            w_o[h0 * DV:(h0 + 1) * DV, :],
            w_o[h1 * DV:(h1 + 1) * DV, :],
        ], axis=0)
        tsl = slice(c * TSH, (c + 1) * TSH)
        in_maps.append({
            "hs_sh": bf(hs_t[:, tsl]),
            "wqa": bf(wqa_pk),
            "wkva": bf(wkva_pk),
            "wkpe": bf(wkpe_pk),
            "wqb": bf(wqb_pk),
            "wkvb": bf(wkvb_pk),
            "wo": bf(wo_c),
            "cosf2": bf(cosf2[:, tsl]),
            "sinf2": bf(sinf2[:, tsl]),
            "perm128": bf(perm128),
            "selswap": bf(selswap),
            "ident": bf(np.eye(128, dtype=np.float32)),
            "maskd": bf(maskd),
            "ones": bf(np.ones((128, 128), dtype=np.float32)),
        })
    return in_maps


def kernel(positions, hidden_states, w_q_a, q_a_ln_w, w_q_b, w_kv_a,
           kv_a_ln_w, w_kv_b, w_o):
    nc = _get_nc()
    in_maps = make_in_maps(positions, hidden_states, w_q_a, q_a_ln_w, w_q_b,
                           w_kv_a, kv_a_ln_w, w_kv_b, w_o)
    res = bass_utils.run_bass_kernel_spmd(nc, in_maps, core_ids=list(range(NCORES)))
    acc = np.zeros((T, HID), dtype=np.float32)
    for c in range(NCORES):
        acc += np.asarray(res.results[c]["out"], dtype=np.float32)
    return acc
